# revision 1
# baseline (speedup 1.0000x reference)
"""Trainium2 Bass kernel for nn_Block_38517266710836.

reference pipeline: channel mixer -> STFT (hann 2048, hop 1024) -> per-frame
recurrence out[f] = (spec[f] + out[f-1]) * transfer -> iSTFT (hann synthesis)
-> overlap-add -> gain -> tanh.

Sharding: 8 cores, data-parallel over (batch, channel-half): core c handles
batch c//2, mixed channels [32*(c%2), +32). Each core receives its batch's
full 64-channel input (the mixer contracts channels) and writes 32 rows.

Per core:
  phase F: corner-turn fused with the mixer via PE transpose-mode matmuls
           (out = x_chunk.T @ mixer_slice); forward windowed DFT as 16x16
           chained [128x128] bf16 matmuls, hann window folded into the
           host-built weights; spec evicted into a scan-layout SBUF tile
           (frames innermost, 17-column chain blocks with col 0 reserved
           for reset/inject).
  phase S: frame recurrence via tensor_tensor_scan (state = T*state + spec,
           op0=mult/op1=add; T-pattern tile has 0 at chain starts for reset,
           in-place sub-scans chained through inject columns), then
           out = T*u as in-place tensor_mul.
  phase I: inverse windowed iDFT (gain folded into weights), overlap-add
           folded into PSUM accumulation, tanh fused into the ScalarE PSUM
           eviction, corner-turn back via PE transposes, DMA out.
"""

import numpy as np

WINDOW = 2048
STEP = 1024
CPD = 64
BATCH = 4
TIME = 65536
FRAMES = 64
NJ = 16              # per-frame time chunks (fwd contraction blocks)
NM = 16              # spectral slot chunks
DSH = 32             # mixed channels per core
GCH = TIME // 128    # 512 global 128-sample chunks
GPAD = GCH + 16      # + zero pad (frame 63 reaches t=66560; extra width so
                     # the forward rhs slice [base, base+2048) stays in-bounds)
FC = 4               # frame chunks for the scan layout
FW = 16              # frames per chunk
CB = 17              # chain block: 1 inject/reset col + 16 frame cols
SPECW = NM * DSH * CB  # 8704 free cols per fc block


def _hann(n):
    return (0.5 - 0.5 * np.cos(2.0 * np.pi * np.arange(n) / n)).astype(np.float64)


def _slot_tables():
    """slot s in [0,2048): s<1024 -> Re[k=s]; s==1024 -> Re[1024] (parked in
    Im[0]'s slot, since Im[0] is identically 0); s>1024 -> Im[k=s-1024]."""
    k_of_slot = np.zeros(2048, np.int64)
    is_im = np.zeros(2048, np.bool_)
    for s in range(2048):
        if s < 1024:
            k_of_slot[s] = s
        elif s == 1024:
            k_of_slot[s] = 1024
        else:
            k_of_slot[s] = s - 1024
            is_im[s] = True
    return k_of_slot, is_im


def build_fwd_weights():
    """[2048 n, 2048 slots]: windowed rfft of one frame, slot layout."""
    n = np.arange(WINDOW, dtype=np.float64)
    w = _hann(WINDOW)
    k_of_slot, is_im = _slot_tables()
    ang = 2.0 * np.pi * np.outer(n, k_of_slot.astype(np.float64)) / WINDOW
    W = np.where(is_im[None, :], -np.sin(ang), np.cos(ang))
    W *= w[:, None]
    return W


def build_inv_weights(gain):
    """[2048 slots, 2048 n]: gain * hann * irfft from slot layout."""
    n = np.arange(WINDOW, dtype=np.float64)
    w = _hann(WINDOW)
    k_of_slot, is_im = _slot_tables()
    ang = 2.0 * np.pi * np.outer(k_of_slot.astype(np.float64), n) / WINDOW
    k = k_of_slot
    re_coef = (2.0 - (k == 0) - (k == 1024))[:, None] / WINDOW * np.cos(ang)
    im_coef = -2.0 / WINDOW * np.sin(ang)
    W = np.where(is_im[:, None], im_coef, re_coef)
    W[1024, :] = np.cos(np.pi * n) / WINDOW
    W *= (gain * w)[None, :]
    return W


def build_t_slots(transfer):
    k_of_slot, _ = _slot_tables()
    return np.asarray(transfer, np.float64)[:, k_of_slot]  # [ch, 2048]


def build_pattern(t_slots_core):
    """T-pattern [128, SPECW]: per (m,d) chain block of CB cols:
    col 0 = 0 (reset/inject), cols 1..16 = T[slot(m,kf), d]."""
    pat = np.zeros((128, SPECW), np.float64)
    for m in range(NM):
        for d in range(DSH):
            base = (m * DSH + d) * CB
            pat[:, base + 1: base + CB] = \
                t_slots_core[d, m * 128:(m + 1) * 128][:, None]
    return pat


def emulate(x, transfer, mixer_matrix, gain, wdtype=np.float32):
    """Numpy emulation of the device math (offline validation)."""
    b, c, t = x.shape
    Wf = build_fwd_weights().astype(wdtype).astype(np.float64)
    Wi = build_inv_weights(float(np.asarray(gain).ravel()[0])).astype(wdtype).astype(np.float64)
    Ts = build_t_slots(transfer)
    y = np.einsum('bct,cd->bdt', np.asarray(x, np.float64),
                  np.asarray(mixer_matrix, np.float64))
    yp = np.pad(y, ((0, 0), (0, 0), (0, STEP)))
    out = np.zeros((b, c, t), np.float64)
    for bi in range(b):
        frames = np.stack([yp[bi, :, f * STEP: f * STEP + WINDOW]
                           for f in range(FRAMES)], 1)
        spec = frames.astype(wdtype).astype(np.float64) @ Wf
        st = np.zeros((c, 2048))
        outs = np.zeros_like(spec)
        for f in range(FRAMES):
            st = (spec[:, f].astype(wdtype).astype(np.float64) + st) * Ts
            outs[:, f] = st
        aud = outs.astype(wdtype).astype(np.float64) @ Wi
        acc = np.zeros((c, t + STEP))
        for f in range(FRAMES):
            acc[:, f * STEP: f * STEP + WINDOW] += aud[:, f]
        out[bi] = np.tanh(acc[:, :t])
    return out.astype(np.float32)


# ---------------------------------------------------------------------------
# Device program
# ---------------------------------------------------------------------------

_CACHED_NC = None


def _build_program():
    import concourse.bacc as bacc
    import concourse.mybir as mybir
    from concourse import tile
    from contextlib import ExitStack

    f32 = mybir.dt.float32
    f32r = mybir.dt.float32r
    bf16 = mybir.dt.bfloat16
    Alu = mybir.AluOpType

    nc = bacc.Bacc("TRN2", target_bir_lowering=False, debug=False, num_devices=8)
    xb = nc.dram_tensor("xb", [CPD, TIME], bf16, kind="ExternalInput").ap()
    mixw = nc.dram_tensor("mixw", [CPD, DSH], bf16, kind="ExternalInput").ap()
    wf = nc.dram_tensor("wf", [NJ * 128, NM * 128], bf16, kind="ExternalInput").ap()
    wi = nc.dram_tensor("wi", [NM * 128, NJ * 128], bf16, kind="ExternalInput").ap()
    patd = nc.dram_tensor("pat", [128, SPECW], bf16, kind="ExternalInput").ap()
    eyed = nc.dram_tensor("eye", [128, 128], f32, kind="ExternalInput").ap()
    eyebd = nc.dram_tensor("eyeb", [128, 128], bf16, kind="ExternalInput").ap()
    yout = nc.dram_tensor("y", [DSH, TIME], f32, kind="ExternalOutput").ap()
    import os
    _dump = os.environ.get("K_DUMP", "")
    dbg = nc.dram_tensor("dbg", [128, FC * SPECW], f32,
                         kind="ExternalOutput").ap() if _dump else None

    XCH = 2048           # x streamed in [64, 2048] chunks (16 g-chunks each)

    with tile.TileContext(nc) as tc:
        with tc.tile_pool(name="persist", bufs=1) as persist:
            spec = persist.tile([128, FC * SPECW], bf16, tag="spec")

            # ================= phase F =================
            with ExitStack() as ctxF:
                wp = ctxF.enter_context(tc.tile_pool(name="wfp", bufs=1))
                wf_t = wp.tile([128, NJ * NM * 128], bf16, tag="wf")
                for j in range(NJ):
                    nc.sync.dma_start(
                        out=wf_t[:, j * NM * 128:(j + 1) * NM * 128],
                        in_=wf[j * 128:(j + 1) * 128, :])
                mx = wp.tile([CPD, DSH], bf16, tag="mx")
                nc.sync.dma_start(out=mx[:], in_=mixw[:])
                eyeb = wp.tile([128, 128], bf16, tag="eyeb")
                nc.sync.dma_start(out=eyeb[:], in_=eyebd[:])
                a_t = wp.tile([128, GPAD * DSH], bf16, tag="a")
                a_t2 = a_t
                nc.vector.memset(a_t[:, GCH * DSH:], 0.0)

                xin = ctxF.enter_context(tc.tile_pool(name="xin", bufs=3))
                ymp = ctxF.enter_context(tc.tile_pool(name="ymp", bufs=2))
                mp = ctxF.enter_context(tc.tile_pool(name="mp", bufs=2, space="PSUM"))
                tp = ctxF.enter_context(tc.tile_pool(name="tp", bufs=2, space="PSUM"))
                sp = ctxF.enter_context(tc.tile_pool(name="sp", bufs=2, space="PSUM"))

                # mixer (col-tiled into 4x32 partitions) then corner-turn:
                # A[128 tfine, (g, d)] bf16
                for xc in range(TIME // XCH):
                    xt = xin.tile([CPD, XCH], bf16, tag="x")
                    nc.sync.dma_start(out=xt[:], in_=xb[:, xc * XCH:(xc + 1) * XCH])
                    pm = mp.tile([128, 512], f32, tag="mix")
                    for q in range(4):
                        nc.tensor.matmul(
                            pm[q * DSH:(q + 1) * DSH, :],
                            mx[:],
                            xt[:, q * 512:(q + 1) * 512],
                            start=True, stop=True,
                            tile_position=(0, q * DSH))
                    ym = ymp.tile([128, 512], bf16, tag="ym")
                    nc.scalar.copy(ym[:], pm[:])
                    # ym[(q,d), tloc]: t = xc*2048 + q*512 + tloc
                    for gq in range(4):  # per 4 g-chunks (one psum turn tile)
                        pt = tp.tile([128, 128], bf16, tag="turn")
                        # one full transpose: ym[(q,d), gq*128 + tf] -> pt[tf, (q,d)]
                        nc.tensor.transpose(
                            pt[:],
                            ym[:, gq * 128: gq * 128 + 128],
                            eyeb[:])
                        # pt[tfine, (q2, d)] covers g = xc*16 + q2*4 + gq
                        g0 = xc * (XCH // 128)
                        dst = a_t[:][:, g0 * DSH:(g0 + 16) * DSH] \
                            .rearrange("p (q2 gq d) -> p q2 gq d", q2=4, gq=4)[
                                :, :, gq, :]
                        psrc = pt[:].rearrange("p (q2 d) -> p q2 d", q2=4)
                        if gq % 2 == 0:
                            nc.scalar.copy(dst, psrc)
                        else:
                            nc.vector.tensor_copy(dst, psrc)

                if _dump == "A":
                    nc.gpsimd.dma_start(out=dbg[:, :GPAD * DSH], in_=a_t[:])

                # forward DFT: frames batched 16 (N=512); m in eighth passes
                for f16 in range(FRAMES // 16):
                    fc = f16
                    for qp in range(8):
                        ps = sp.tile([128, 1024], f32, tag="sm")
                        for mi in range(2):
                            m = qp * 2 + mi
                            out_ap = ps[:][:, mi * 512:(mi + 1) * 512] \
                                .rearrange("p (d f) -> p f d", f=16)
                            for j in range(NJ):
                                base = (128 * f16 + j) * DSH
                                rhs = a_t[:][:, base: base + 4096] \
                                    .rearrange("p (f q) -> p f q", f=16)[:, :, :DSH]
                                nc.tensor.matmul(
                                    out_ap,
                                    wf_t[:, (j * NM + m) * 128:(j * NM + m + 1) * 128],
                                    rhs,
                                    start=(j == 0), stop=(j == NJ - 1))
                        for mi in range(2):
                            m = qp * 2 + mi
                            src = ps[:][:, mi * 512:(mi + 1) * 512] \
                                .rearrange("p (d f) -> p d f", f=16)
                            doff = fc * SPECW + m * DSH * CB
                            dst = spec[:][:, doff: doff + DSH * CB] \
                                .rearrange("p (d c) -> p d c", c=CB)[:, :, 1: 1 + FW]
                            if mi % 2 == 0:
                                nc.scalar.copy(dst, src)
                            else:
                                nc.vector.tensor_copy(dst, src)

            if _dump == "F":
                with tc.tile_pool(name="dbgp", bufs=1) as dp:
                    dt_ = dp.tile([128, FC * SPECW], f32, tag="dbg")
                    nc.vector.tensor_copy(dt_[:], spec[:])
                    nc.sync.dma_start(out=dbg, in_=dt_[:])

            # ================= phase S =================
            with ExitStack() as ctxS:
                spl = ctxS.enter_context(tc.tile_pool(name="spl", bufs=1))
                pat = spl.tile([128, SPECW], bf16, tag="pat")
                nc.sync.dma_start(out=pat[:], in_=patd[:])
                # chain col 0 of the first block must read as 0 (fresh state)
                nc.vector.memset(
                    spec[:][:, 0:SPECW].rearrange(
                        "p (md c) -> p md c", c=CB)[:, :, 0:1], 0.0)
                for fc in range(FC):
                    if fc > 0:
                        src = spec[:][:, (fc - 1) * SPECW: fc * SPECW] \
                            .rearrange("p (md c) -> p md c", c=CB)[:, :, CB - 1: CB]
                        dst = spec[:][:, fc * SPECW: (fc + 1) * SPECW] \
                            .rearrange("p (md c) -> p md c", c=CB)[:, :, 0:1]
                        nc.vector.tensor_copy(dst, src)
                    nc.vector.tensor_tensor_scan(
                        spec[:, fc * SPECW:(fc + 1) * SPECW],
                        pat[:],
                        spec[:, fc * SPECW:(fc + 1) * SPECW],
                        0.0, Alu.mult, Alu.add)
                for fc in range(FC):
                    nc.vector.tensor_mul(
                        spec[:, fc * SPECW:(fc + 1) * SPECW],
                        spec[:, fc * SPECW:(fc + 1) * SPECW],
                        pat[:])

            if _dump == "S":
                with tc.tile_pool(name="dbgp", bufs=1) as dp:
                    dt_ = dp.tile([128, FC * SPECW], f32, tag="dbg")
                    nc.vector.tensor_copy(dt_[:], spec[:])
                    nc.sync.dma_start(out=dbg, in_=dt_[:])

            # ================= phase I =================
            with ExitStack() as ctxI:
                wp2 = ctxI.enter_context(tc.tile_pool(name="wip", bufs=1))
                wi_t = wp2.tile([128, NM * NJ * 128], bf16, tag="wi")
                for m in range(NM):
                    nc.sync.dma_start(
                        out=wi_t[:, m * NJ * 128:(m + 1) * NJ * 128],
                        in_=wi[m * 128:(m + 1) * 128, :])
                eye = wp2.tile([128, 128], f32, tag="eye")
                nc.sync.dma_start(out=eye[:], in_=eyed[:])

                op = ctxI.enter_context(tc.tile_pool(name="ola", bufs=2, space="PSUM"))
                t4 = ctxI.enter_context(tc.tile_pool(name="t4", bufs=2, space="PSUM"))
                tout = ctxI.enter_context(tc.tile_pool(name="tout", bufs=2))
                stg = ctxI.enter_context(tc.tile_pool(name="stg", bufs=3))

                yv = yout.rearrange("d (a4 fl t) -> fl d a4 t", fl=4, t=1024)

                for fc in range(FC):
                    for rp in range(4):
                        for ji in range(2):
                            j = rp * 2 + ji
                            ps = op.tile([128, FW * DSH], f32, tag=f"ola{ji}")
                            out_full = ps[:].rearrange("p (f d) -> p d f", f=FW)
                            # set A: frames 16fc+fi, chunk j
                            for m in range(NM):
                                base = fc * SPECW + m * DSH * CB
                                rhs = spec[:][:, base: base + DSH * CB] \
                                    .rearrange("p (d c) -> p d c", c=CB)[:, :, 1: 1 + FW]
                                nc.tensor.matmul(
                                    out_full,
                                    wi_t[:, (m * NJ + j) * 128:(m * NJ + j + 1) * 128],
                                    rhs, start=(m == 0), stop=False)
                            # set B: frames 16fc+fi-1 (fi>=1), chunk j+8
                            for m in range(NM):
                                base = fc * SPECW + m * DSH * CB
                                rhs = spec[:][:, base: base + DSH * CB] \
                                    .rearrange("p (d c) -> p d c", c=CB)[:, :, 1: FW]
                                nc.tensor.matmul(
                                    out_full[:, :, 1:],
                                    wi_t[:, (m * NJ + j + 8) * 128:(m * NJ + j + 8 + 1) * 128],
                                    rhs, start=False,
                                    stop=(fc == 0 and m == NM - 1))
                            # boundary: fi=0 gets frame 16fc-1 (chunk j+8)
                            if fc > 0:
                                for m in range(NM):
                                    base = (fc - 1) * SPECW + m * DSH * CB + CB - 1
                                    rhs = spec[:][:, base: base + DSH * CB] \
                                        .rearrange("p (d c) -> p d c", c=CB)[:, :, 0:1]
                                    nc.tensor.matmul(
                                        out_full[:, :, 0:1],
                                        wi_t[:, (m * NJ + j + 8) * 128:(m * NJ + j + 8 + 1) * 128],
                                        rhs, start=False, stop=(m == NM - 1))
                            # tanh eviction
                            tt = tout.tile([128, FW * DSH], f32, tag=f"to{ji}")
                            nc.scalar.activation(
                                tt[:], ps[:], mybir.ActivationFunctionType.Tanh)
                            # corner-turn back + store
                            p4 = t4.tile([128, 512], f32, tag="t4")
                            for r2 in range(4):
                                nc.tensor.transpose(
                                    p4[:, r2 * 128:(r2 + 1) * 128],
                                    tt[:, r2 * 128:(r2 + 1) * 128],
                                    eye[:])
                            st = stg.tile([128, 512], f32, tag="stg")
                            if ji == 0:
                                nc.vector.tensor_copy(st[:], p4[:])
                            else:
                                nc.scalar.copy(st[:], p4[:])
                            for r2 in range(4):
                                dst = yv[:, :, 4 * fc + r2, j * 128:(j + 1) * 128]
                                nc.sync.dma_start(
                                    out=dst,
                                    in_=st[:, r2 * 128:(r2 + 1) * 128])
    nc.compile()
    return nc


def _get_nc():
    global _CACHED_NC
    if _CACHED_NC is None:
        _CACHED_NC = _build_program()
    return _CACHED_NC


def kernel(x, transfer, mixer_matrix, gain, _trace=False):
    import ml_dtypes
    from concourse.bass_utils import run_bass_kernel_spmd

    x = np.ascontiguousarray(np.asarray(x, np.float32))
    transfer = np.asarray(transfer, np.float32)
    mixer_matrix = np.asarray(mixer_matrix, np.float32)
    gain = np.asarray(gain, np.float32)

    bf = ml_dtypes.bfloat16
    Wf = build_fwd_weights()
    Wi = build_inv_weights(float(gain.ravel()[0]))
    # [NJ*128, NM*128] row-block j col-block m = lhsT [n-fine, slot-fine]
    wf_np = Wf.astype(bf)
    wi_np = Wi.astype(bf)
    Ts = build_t_slots(transfer)
    eye = np.eye(128, dtype=np.float32)
    eyeb = np.eye(128, dtype=np.float64).astype(bf)

    in_maps = []
    for c in range(8):
        b, dh = c // 2, c % 2
        mixw = mixer_matrix[:, dh * DSH:(dh + 1) * DSH].astype(bf)
        patc = build_pattern(Ts[dh * DSH:(dh + 1) * DSH]).astype(bf)
        in_maps.append({
            "xb": x[b].astype(bf),
            "mixw": mixw,
            "wf": wf_np,
            "wi": wi_np,
            "pat": patc,
            "eye": eye,
            "eyeb": eyeb,
        })

    nc = _get_nc()
    res = run_bass_kernel_spmd(nc, in_maps, list(range(8)), trace=_trace)
    out = np.zeros((BATCH, CPD, TIME), np.float32)
    for c in range(8):
        b, dh = c // 2, c % 2
        out[b, dh * DSH:(dh + 1) * DSH] = res.results[c]["y"]
    import os
    if os.environ.get("K_DUMP", ""):
        np.save("/tmp/dbg0.npy", res.results[0]["dbg"])
    if _trace:
        return out, res
    return out

